# revision 34
# baseline (speedup 1.0000x reference)
"""Quantized matmul (uint4 groupwise dequant) on 8 Trainium2 NeuronCores.

Computes out = a_f32 @ W where W[k, n] = (q[k, n] - zeros[k//128, n]) * scales[k//128, n].

Sharding: tensor-parallel along N (output features). Each of the 8 cores gets
N_LOCAL = 512 columns of q/scales/zeros and the full `a` (replicated). Each
core dequantizes its W slice to fp16 once into SBUF, then runs a dense
fp16 matmul with fp32 PSUM accumulation.

Device kernel layout choices (all host-side prep is pure layout/sharding):
 - `a` is fed pre-transposed and tiled as aT[m_out, k_in, k_out*128 + m_in]
   so each [128, 4096] SBUF tile is one contiguous 1 MiB DMA and slices
   [:, k*128:(k+1)*128] are matmul lhsT tiles (K on partitions).
 - q values are 0..15, so the int32 container is narrowed to int8 on the
   host (lossless) to quarter its DMA cost.
 - zeros/scales are replicated across partitions by stride-0 DRAM->SBUF
   DMAs. A partition-broadcast costs ~4-7us of DMA-ring time almost
   independent of its size (128 descriptor rows dominate), so they are
   issued as just EIGHT 8-group broadcasts spread over all four sequencer
   queues' rings: the first round (groups 0..15) lands ~14us, the second
   round reuses the tiles and lands long before the wavefront needs k>=16.

Schedule: the PE warms up with 7 dummy matmuls (the HAM clock gate needs
~3.4us of activity to reach 8/8 = 2.4 GHz), then runs a single
availability-ordered wavefront covering m-tiles 0..13: aT quarters for
m0/m1 give the PE work in the first microseconds, and the extension to 14
m-tiles (PSUM banks recycled with explicit ordering keys) gives the
in-order PE stream enough backlog to absorb the W-dequant trickle without
going idle. Remaining m-tiles run m-outer/k-inner with inline epilogues;
the last epilogue is split across ACT+DVE and two DMA queues.
"""

import numpy as np

M, K, N = 4096, 4096, 4096
G = 128          # quant group size
P = 128          # partitions
NCORES = 8
NL = N // NCORES          # 512 output columns per core
KT = K // P               # 32 k tiles (== quant groups)
MT = M // P               # 32 m tiles
NQM = 2                   # m-tiles loaded as quarters (m0, m1)
NFULL = 6                 # early full m-tiles (m2..m7)
WAVE = 14                 # m-tiles in the availability-sorted wavefront
AQ = 4                    # quarters per quartered m-tile
WARMUP = 7
BCG = 8                   # groups per zs broadcast
QCG = 4                   # groups per q chunk

_CACHE = {}


def _build_nc():
    import concourse.bacc as bacc
    import concourse.mybir as mybir
    import concourse.tile as tile
    from concourse.bass import ts

    f16 = mybir.dt.float16
    f32 = mybir.dt.float32
    i8 = mybir.dt.int8

    nc = bacc.Bacc("TRN2", target_bir_lowering=False, debug=False)

    aT = nc.dram_tensor("aT", [MT, P, K], f16, kind="ExternalInput").ap()
    q = nc.dram_tensor("q", [KT, P, NL], i8, kind="ExternalInput").ap()
    zsm = nc.dram_tensor("zsm", [1, KT * NL], f16, kind="ExternalInput").ap()
    ssm = nc.dram_tensor("ssm", [1, KT * NL], f16, kind="ExternalInput").ap()
    out = nc.dram_tensor("out", [MT, P, NL], f32, kind="ExternalOutput").ap()

    with tile.TileContext(nc) as tc:
        # Availability model (us, relative to DMA boot).
        RATE = 0.358      # MB/us of HBM bandwidth for bulk streams
        DVE_G = 0.95      # us of dequant work per group per engine
        BC_BASE = [8.0, 19.0, 26.0, 32.0]  # zs broadcast round arrivals

        avail_w = [0.0] * KT
        avail_aq = {}
        avail_at = [0.0] * WAVE

        with (
            tc.tile_pool(name="w", bufs=KT) as wpool,
            tc.tile_pool(name="zsb", bufs=1) as zsbpool,
            tc.tile_pool(name="qraw", bufs=KT // QCG) as qpool,
            tc.tile_pool(name="deq", bufs=4) as dqpool,
            tc.tile_pool(name="a0", bufs=NQM * AQ) as a0pool,
            tc.tile_pool(name="atb", bufs=NFULL) as bpool,
            tc.tile_pool(name="at", bufs=6) as apool,
            tc.tile_pool(name="ot", bufs=2) as opool,
            tc.tile_pool(name="ps", bufs=8, space="PSUM") as pspool,
        ):
            cum_mb = 0.0
            aqt = {}
            ats = [None] * WAVE
            w_tiles = [None] * KT

            # PE warm-up: dummy matmuls pull the HAM clock gate to 8/8.
            warm_in = dqpool.tile([P, NL], f16, name="warm_in", tag="d")
            nc.gpsimd.memset(warm_in[:], 0.0)
            warm_ps = pspool.tile([P, NL], f32, name="warm_ps", tag="ps")
            for i in range(WARMUP):
                nc.tensor.matmul(
                    warm_ps[:],
                    warm_in[:, 0:P],
                    warm_in[:],
                    start=(i == 0),
                    stop=(i == WARMUP - 1),
                )

            # --- zs broadcasts: 4 tiles, 4 rounds of 8 groups. The z
            # rounds all ride the Scalar ring; the first s round rides the
            # Sync ring (ahead of the aT stream) and the rest the GpSimd
            # ring behind the q chunks. Ring/tile reuse is timed so a
            # trigger only ever waits for dequants that have already run.
            def bc_slice(r):
                return slice(r * BCG * NL, (r + 1) * BCG * NL)

            zbc = [None, None]  # round-parity tiles: rounds 0,2 / 1,3
            sbc = [None, None]
            zbc[0] = zsbpool.tile([P, BCG * NL], f16, name="zbcA", tag="zA")
            nc.scalar.dma_start(zbc[0][:], zsm[:, bc_slice(0)].partition_broadcast(P))
            sbc[0] = zsbpool.tile([P, BCG * NL], f16, name="sbcA", tag="sA")
            nc.sync.dma_start(sbc[0][:], ssm[:, bc_slice(0)].partition_broadcast(P))
            zbc[1] = zsbpool.tile([P, BCG * NL], f16, name="zbcB", tag="zB")
            nc.scalar.dma_start(zbc[1][:], zsm[:, bc_slice(1)].partition_broadcast(P))
            sbc[1] = zsbpool.tile([P, BCG * NL], f16, name="sbcB", tag="sB")

            # --- q chunks: 8 flat DMAs on the GpSimd (SWDGE) queue, then
            # the first gpsimd-ring s broadcast.
            qts = []
            for j in range(KT // QCG):
                qt = qpool.tile([P, QCG, NL], i8, name=f"qt{j}", tag="qt")
                nc.gpsimd.dma_start(
                    qt[:],
                    q[j * QCG : (j + 1) * QCG].rearrange("g p n -> p g n"),
                )
                qts.append(qt)
            q_mb = (KT * P * NL) / 1e6
            nc.gpsimd.dma_start(sbc[1][:], ssm[:, bc_slice(1)].partition_broadcast(P))

            # --- a-side loads on the Sync queue (cum model includes the
            # zs first round + q stream as concurrent background traffic).
            cum_mb += 2 * BCG * NL * P * 2 / 1e6 * 2 + q_mb

            def emit_quarter(mi, v):
                nonlocal cum_mb
                t = a0pool.tile([P, K // AQ], f16, name=f"aq{mi}_{v}", tag="a0")
                nc.sync.dma_start(t[:], aT[mi][:, ts(v, K // AQ)])
                cum_mb += (P * K // AQ) * 2 / 1e6
                avail_aq[(mi, v)] = cum_mb / RATE
                aqt[(mi, v)] = t

            def emit_at(mi):
                nonlocal cum_mb
                pool = bpool if mi < NQM + NFULL else apool
                t = pool.tile([P, K], f16, name=f"at_{mi}", tag="at")
                nc.sync.dma_start(t[:], aT[mi])
                cum_mb += (P * K) * 2 / 1e6
                avail_at[mi] = cum_mb / RATE
                ats[mi] = t

            for v in range(AQ):
                for mi in range(NQM):
                    emit_quarter(mi, v)
            for mi in range(NQM, WAVE):
                emit_at(mi)

            # --- dequant: even groups on DVE, odd on GpSimd, emitted in
            # an order that lets each queue's round-2 broadcast trigger
            # fire without stalling behind still-blocked compute.
            def emit_dequant(k):
                rnd = k // BCG
                par = rnd % 2
                off = (k % BCG) * NL
                zap = zbc[par][:, off : off + NL]
                sap = sbc[par][:, off : off + NL]
                qt = qts[k // QCG][:, k % QCG, :]
                eng = nc.vector if k % 2 == 0 else nc.gpsimd
                d = dqpool.tile([P, NL], f16, tag="d")
                eng.tensor_sub(out=d[:], in0=qt, in1=zap)
                wt = wpool.tile([P, NL], f16, tag="w")
                eng.tensor_mul(out=wt[:], in0=d[:], in1=sap)
                w_tiles[k] = wt
                base = BC_BASE[rnd]
                ei = k % 2
                ef[ei] = max(ef[ei], base, (q_mb * (k // QCG + 1) / 8) / RATE) + DVE_G
                avail_w[k] = ef[ei]

            ef = [BC_BASE[0], BC_BASE[0]]
            for k in range(BCG):
                emit_dequant(k)
            # later rounds reuse the tiles: each trigger's ring/tile wait
            # is on dequants already emitted above it on the same queue.
            nc.scalar.dma_start(zbc[0][:], zsm[:, bc_slice(2)].partition_broadcast(P))
            nc.gpsimd.dma_start(sbc[0][:], ssm[:, bc_slice(2)].partition_broadcast(P))
            for k in range(BCG, 2 * BCG):
                emit_dequant(k)
            nc.scalar.dma_start(zbc[1][:], zsm[:, bc_slice(3)].partition_broadcast(P))
            nc.gpsimd.dma_start(sbc[1][:], ssm[:, bc_slice(3)].partition_broadcast(P))
            for k in range(2 * BCG, KT):
                emit_dequant(k)

            def lhsT(mi, k):
                if mi < NQM:
                    return aqt[(mi, k * AQ // KT)][:, ts(k % (KT // AQ), P)]
                return ats[mi][:, ts(k, P)]

            def avail_lhs(mi, k):
                if mi < NQM:
                    return avail_aq[(mi, k * AQ // KT)]
                return avail_at[mi]

            # Wavefront over m0..WAVE-1, ordered by modeled availability.
            # Keys are prefix-maxed per m (k==0 carries start=True and must
            # go first) and chained across PSUM-bank reuse.
            pss = [
                pspool.tile([P, NL], f32, name=f"ps0_{i}", tag="ps")
                for i in range(WAVE)
            ]
            keys = {}
            for mi in range(WAVE):
                run = keys[(mi - 8, KT - 1)] if mi >= 8 else 0.0
                for k in range(KT):
                    run = max(run, avail_lhs(mi, k), avail_w[k])
                    keys[(mi, k)] = run
            order = sorted(
                ((mi, k) for mi in range(WAVE) for k in range(KT)),
                key=lambda t: (keys[t], t[0], t[1]),
            )
            for mi, k in order:
                nc.tensor.matmul(
                    pss[mi][:],
                    lhsT(mi, k),
                    w_tiles[k][:],
                    start=(k == 0),
                    stop=(k == KT - 1),
                )
            for mi in sorted(range(WAVE), key=lambda m: keys[(m, KT - 1)]):
                ot = opool.tile([P, NL], f32)
                nc.scalar.copy(ot[:], pss[mi][:])
                nc.scalar.dma_start(out[mi], ot[:])

            # Remaining m-tiles: m-outer, k-inner, inline epilogue.
            for m in range(WAVE, MT):
                at = apool.tile([P, K], f16, name=f"at_{m}", tag="at")
                nc.sync.dma_start(at[:], aT[m])
                ps = pspool.tile([P, NL], f32, name=f"ps_{m}", tag="ps")
                for k in range(KT):
                    nc.tensor.matmul(
                        ps[:],
                        at[:, ts(k, P)],
                        w_tiles[k][:],
                        start=(k == 0),
                        stop=(k == KT - 1),
                    )
                if m < MT - 1:
                    ot = opool.tile([P, NL], f32)
                    nc.scalar.copy(ot[:], ps[:])
                    nc.scalar.dma_start(out[m], ot[:])
                else:
                    # tail: split the last epilogue across ACT+DVE and two
                    # DMA queues.
                    h = NL // 2
                    ota = opool.tile([P, h], f32)
                    otb = opool.tile([P, h], f32)
                    nc.scalar.copy(ota[:], ps[:, :h])
                    nc.vector.tensor_copy(otb[:], ps[:, h:])
                    nc.scalar.dma_start(out[m][:, :h], ota[:])
                    nc.sync.dma_start(out[m][:, h:], otb[:])

    nc.compile()
    return nc


def _shard_inputs(a, q_weight, scales, zeros):
    """Host-side shard/layout. Pure slicing, transposition and replication."""
    # aT[m_out, k_in, k_out*128 + m_in] = a[m_out*128 + m_in, k_out*128 + k_in]
    aT = np.ascontiguousarray(
        a.reshape(MT, P, KT, P).transpose(0, 3, 2, 1)
    ).reshape(MT, P, K)
    # q values are 0..15: int8 container is lossless.
    q8 = q_weight.astype(np.int8)

    in_maps = []
    for c in range(NCORES):
        sl = slice(c * NL, (c + 1) * NL)
        q_c = np.ascontiguousarray(q8[:, sl]).reshape(KT, P, NL)
        z_c = np.ascontiguousarray(zeros[:, sl]).reshape(1, KT * NL)
        s_c = np.ascontiguousarray(scales[:, sl]).reshape(1, KT * NL)
        in_maps.append({"aT": aT, "q": q_c, "zsm": z_c, "ssm": s_c})
    return in_maps


def _run(inputs, trace=False):
    from concourse import bass_utils

    if "nc" not in _CACHE:
        _CACHE["nc"] = _build_nc()
    nc = _CACHE["nc"]

    a = np.asarray(inputs["a"], dtype=np.float16)
    q_weight = np.asarray(inputs["q_weight"], dtype=np.int32)
    scales = np.asarray(inputs["scales"], dtype=np.float16)
    zeros = np.asarray(inputs["zeros"], dtype=np.float16)

    in_maps = _shard_inputs(a, q_weight, scales, zeros)
    res = bass_utils.run_bass_kernel_spmd(
        nc, in_maps, core_ids=list(range(NCORES)), trace=trace
    )

    out = np.empty((M, N), dtype=np.float32)
    for c in range(NCORES):
        out[:, c * NL : (c + 1) * NL] = res.results[c]["out"].reshape(M, NL)
    return out, res


def kernel(**inputs) -> np.ndarray:
    out, _ = _run(inputs, trace=False)
    return out
